# revision 38
# baseline (speedup 1.0000x reference)
"""Causal self-attention kernel for 8 Trainium2 NeuronCores.

Problem (hardcoded): x [4, 2048, 768] f32, W [768, 2304] f32, b [2304] f32.
reference: qkv = x@W+b; 8 heads, head_dim 96; causal softmax attention.

Sharding: core c handles batch c//2 and heads 4*(c%2) .. 4*(c%2)+3
(data-parallel over batch x tensor-parallel over heads). Host shards
inputs / gathers outputs around one SPMD NEFF; no device collectives.

v2 design (all matmul inputs bf16):
  - qk projection "packed": the core's 768 q|k output features are computed
    transposed in 6 full-128-row PSUM groups (vs 8 at 96 rows), then a
    12-op DVE evacuation (with per-partition bias) unpacks group rows into
    per-head qT/kT SBUF tiles [96, 2, 512].
  - v projection natural [seq, 4h, 96+1] with a ones column so the PV
    matmul also produces softmax denominators (bias via K=1 ones matmul).
  - attention in S^T layout, j-blocks of 128 k processed in groups of 4:
    4 matmuls -> one [128, <=2048] bf16 PSUM tile -> ONE wide exp ->
    (diag: 4 gpsimd causal mask selects) -> 4 PV matmuls accumulating
    o^T[d+1, 512] in fp32 PSUM. Diagonal group packs widths 512/384/256/128
    at offsets 0/512/1024/1280 so the exp covers 1408 cols, not 2048.
  - output: o^T PE-transposed back to natural [q, d] via bf16 transposes
    into one [128, 4, 128] PSUM tile, scaled by 1/denominator, DMA out.
  - emission interleaves projection units of block nb with attention units
    of block nb-1 so the scalar engine (exp) stays fed while the PE does
    projection work.
"""

import functools
from contextlib import ExitStack

import numpy as np

import concourse.bacc as bacc
import concourse.bass as bass
import concourse.mybir as mybir
import concourse.tile as tile
from concourse.bass_utils import run_bass_kernel_spmd
from concourse.masks import make_identity

F32 = mybir.dt.float32
F32R = mybir.dt.float32r
BF16 = mybir.dt.bfloat16

B, N, C, H = 4, 2048, 768, 8
D = C // H            # 96
NCORES = 8
LH = 4                # local heads per core
KC = C // 128         # 6 contraction chunks
NB = N // 512         # 4 seq blocks of 512
QKF = 2 * LH * D      # 768 packed q|k features per core
QKG = QKF // 128      # 6 psum groups
OUTC = LH * D         # 384
SCALE = float(1.0 / np.sqrt(np.float32(D)))
UNROLL2 = False
USE_F32R = False      # kept for compat; v2 is bf16-only
MASK_ENGINE = "gpsimd"
ORDER = "proj_first"  # proj_first | attn_first
PV_TRAIL = 2          # PV lags S/exp by this many j-groups

# attention processes j-blocks (128 k rows) in groups of 2 per [128, 1024]
# f32 PSUM tile (2 banks); each matmul output stays inside one 2KB bank.
# Each group entry: (j_rel_a, free_off, width); diagonal groups pack widths
# 512/384 and 256/128 tightly so the exp covers 896 / 384 cols.


# Packed qk projection: per half t (0=q, 1=k) the core's 4x96 features are
# computed in 3 full-128-row PSUM groups. Hardware partition-AP rule: an AP
# may start only at a 32-multiple and must not cross the next 64-boundary
# unless it starts at 0 (so (32, span>32) and (96, span>32) are illegal).
# The evacuation segments below are all legal because heads 1's rows are
# stored rotated (head-rows 0:32 at tile rows 64:96, head-rows 32:96 at tile
# rows 0:64) — any consistent permutation of the d-contraction is valid as
# long as q and k use the same one (they do: both halves share this layout).
# Segment: (group_offset, src_p0, src_p1, head, dst_r0); W columns are packed
# host-side in matching order (see shard_inputs).
QK_SEGMENTS_T = [
    (0, 0, 96, 0, 0),
    (0, 96, 128, 1, 64),
    (1, 0, 64, 1, 0),
    (1, 64, 128, 2, 0),
    (2, 0, 96, 3, 0),
    (2, 96, 128, 2, 64),
]

# per-head W column slices (r0, r1) in packed order for one t-half
QK_COL_ORDER = [(0, 0, 96), (1, 0, 32), (1, 32, 96), (2, 0, 64),
                (3, 0, 96), (2, 64, 96)]


@functools.lru_cache(maxsize=4)
def build(reps=1, use_f32r=False, inline2=False):
    assert not use_f32r, "v2 kernel is bf16-only"
    MDT = BF16
    nc = bacc.Bacc("TRN2", target_bir_lowering=False, debug=False,
                   num_devices=NCORES)
    xt_d = nc.dram_tensor("xt", [C, N], MDT, kind="ExternalInput")
    wqk_d = nc.dram_tensor("wqk", [C, QKF], MDT, kind="ExternalInput")
    wv_d = nc.dram_tensor("wv", [C, LH * D], MDT, kind="ExternalInput")
    bqk_d = nc.dram_tensor("bqk", [128, QKG], F32, kind="ExternalInput")
    bv_d = nc.dram_tensor("bv", [1, LH * D], MDT, kind="ExternalInput")
    out_d = nc.dram_tensor("out", [N, OUTC], F32, kind="ExternalOutput")

    xt_v = xt_d.ap().rearrange("(kc p) n -> p kc n", p=128)
    wqk_v = wqk_d.ap().rearrange("(kc p) m -> p kc m", p=128)
    wv_v = wv_d.ap().rearrange("(kc p) m -> p kc m", p=128)
    out_v = out_d.ap().rearrange("(qq t p) c -> qq p t c", t=4, p=128)

    with tile.TileContext(nc) as tc, ExitStack() as ctx:
        const = ctx.enter_context(tc.tile_pool(name="const", bufs=1))
        wpool = ctx.enter_context(tc.tile_pool(name="w", bufs=1))
        xpool = ctx.enter_context(tc.tile_pool(name="x", bufs=5))
        qkpool = ctx.enter_context(tc.tile_pool(name="qk", bufs=NB + 1))
        vpool = ctx.enter_context(tc.tile_pool(name="vaug", bufs=NB + 1))
        ppool = ctx.enter_context(tc.tile_pool(name="p", bufs=4))
        opool = ctx.enter_context(tc.tile_pool(name="osb", bufs=3))
        rpool = ctx.enter_context(tc.tile_pool(name="r", bufs=4))
        spool = ctx.enter_context(tc.tile_pool(name="stage", bufs=3))
        ps_proj = ctx.enter_context(
            tc.tile_pool(name="ps_proj", bufs=2, space="PSUM"))
        ps_s = ctx.enter_context(
            tc.tile_pool(name="ps_s", bufs=2, space="PSUM"))
        ps_o = ctx.enter_context(
            tc.tile_pool(name="ps_o", bufs=2, space="PSUM"))
        ps_t = ps_proj  # transposes share the proj banks (same tag "proj")

        # one-time constants
        identity = const.tile([128, 128], MDT)
        make_identity(nc, identity[:])
        ones = const.tile([1, 128], F32)
        nc.gpsimd.memset(ones[:], 1.0)
        ones_r = const.tile([1, 128], MDT)
        nc.vector.tensor_copy(ones_r[:], ones[:])
        vones = const.tile([128, 4, LH, 1], F32)
        nc.gpsimd.memset(vones[:], 1.0)

        stage_ref = [None]
        wqk_sb = wpool.tile([128, KC, QKF], MDT, tag="wqk")
        wv_sb = wpool.tile([128, KC, LH * D], MDT, tag="wv")
        for kc in range(KC):
            nc.sync.dma_start(wqk_sb[:, kc, :], wqk_v[:, kc, :])
            nc.sync.dma_start(wv_sb[:, kc, :], wv_v[:, kc, :])
        bqk_sb = wpool.tile([128, QKG], F32, tag="bqk")
        nc.sync.dma_start(bqk_sb[:], bqk_d.ap())
        bv_sb = wpool.tile([1, LH * D], MDT, tag="bv")
        nc.sync.dma_start(bv_sb[:], bv_d.ap())

        def emit_xt(nb):
            xt_sb = xpool.tile([128, KC, 512], MDT, tag="xt")
            for kc in range(KC):
                nc.sync.dma_start(
                    xt_sb[:, kc, :], xt_v[:, kc, nb * 512:(nb + 1) * 512])
            return xt_sb

        def emit_vproj(xt_sb, va, mt):
            vps = ps_proj.tile([128, 512], F32, tag="proj")
            nc.tensor.matmul(vps[:, 0:LH * D], ones_r[:, :], bv_sb[:, :],
                             start=True, stop=False)
            for kc in range(KC):
                nc.tensor.matmul(
                    vps[:, 0:LH * D],
                    xt_sb[:, kc, mt * 128:(mt + 1) * 128],
                    wv_sb[:, kc, :],
                    start=False, stop=(kc == KC - 1))
            nc.vector.tensor_copy(
                va[:, mt, :, 0:D],
                vps[:, 0:LH * D].rearrange("p (h d) -> p h d", h=LH))

        def emit_qkg(xt_sb, qk_h, g):
            qps = ps_proj.tile([128, 512], F32, tag="proj")
            for kc in range(KC):
                nc.tensor.matmul(
                    qps[:, :],
                    wqk_sb[:, kc, 128 * g:128 * g + 128],
                    xt_sb[:, kc, :],
                    start=(kc == 0), stop=(kc == KC - 1))
            t, grel = divmod(g, QKG // 2)
            for (go, p0, p1, h, r0) in QK_SEGMENTS_T:
                if go != grel:
                    continue
                nc.vector.tensor_scalar_add(
                    qk_h[h][r0:r0 + (p1 - p0), t, :],
                    qps[p0:p1, :],
                    bqk_sb[p0:p1, g:g + 1])

        def emit_pv(ops, h, ptinfo, va_all, first, last):
            pt, entries, diag = ptinfo
            n = len(entries)
            for i, (j, off, w) in enumerate(entries):
                qcol = 128 * (j % 4) if diag else 0
                nc.tensor.matmul(
                    ops[:, qcol:512],
                    va_all[j // 4][:, j % 4, h, :],
                    pt[:, off:off + w],
                    start=(first and i == 0), stop=(last and i == n - 1))

        def emit_attn(h, Q, qk_all, va_all, carry=None):
            ops = ps_o.tile([D + 1, 512], F32, tag="o")
            # diagonal groups first: their exp -> mask -> PV latency then
            # hides under the off-diagonal groups' matmuls
            groups = [
                (True, [(4 * Q, 0, 512), (4 * Q + 1, 512, 384)]),
                (True, [(4 * Q + 2, 0, 256), (4 * Q + 3, 256, 128)]),
            ]
            for g in range(Q):
                groups.append((False, [(4 * g, 0, 512), (4 * g + 1, 512, 512)]))
                groups.append((False, [(4 * g + 2, 0, 512),
                                       (4 * g + 3, 512, 512)]))
            pts = []
            for gi, (diag, entries) in enumerate(groups):
                sps = ps_s.tile([128, 1024], F32, tag="s")
                for (j, off, w) in entries:
                    qrel = 128 * (j % 4) if diag else 0
                    nc.tensor.matmul(
                        sps[:, off:off + w],
                        qk_all[j // 4][h][:, 1, 128 * (j % 4):
                                          128 * (j % 4) + 128],
                        qk_all[Q][h][:, 0, qrel:512],
                        start=True, stop=True)
                jl, offl, wl = entries[-1]
                ew = offl + wl
                pt = ppool.tile([128, 1024], MDT, tag="p")
                nc.scalar.activation(
                    pt[:, 0:ew], sps[:, 0:ew],
                    mybir.ActivationFunctionType.Exp, scale=SCALE)
                if diag:
                    for (j, off, w) in entries:
                        nc.gpsimd.affine_select(
                            out=pt[:, off:off + 128],
                            in_=pt[:, off:off + 128],
                            compare_op=mybir.AluOpType.is_ge,
                            fill=0.0, base=0, pattern=[[1, 128]],
                            channel_multiplier=-1)
                pts.append((pt, entries, diag))
                if gi == 0 and carry is not None:
                    carry()  # previous unit's output path, behind our S(0)
                if gi >= PV_TRAIL:
                    emit_pv(ops, h, pts[gi - PV_TRAIL], va_all,
                            first=(gi - PV_TRAIL == 0), last=False)
            ng = len(groups)
            for gi in range(max(0, ng - PV_TRAIL), ng):
                emit_pv(ops, h, pts[gi], va_all,
                        first=(gi == 0), last=(gi == ng - 1))

            # ---- output: transpose, normalize into the per-Q staging ----
            o_sb = opool.tile([D + 1, 512], MDT, tag="osb")
            nc.vector.tensor_copy(o_sb[:], ops[:])
            stage = stage_ref[0]

            def output_path():
                tps = ps_t.tile([128, 4, 128], MDT, tag="proj")
                for t in range(4):
                    nc.tensor.transpose(
                        tps[:, t, 0:D + 1], o_sb[:, t * 128:(t + 1) * 128],
                        identity[0:D + 1, 0:D + 1])
                rr = rpool.tile([128, 4], F32, tag="r")
                nc.vector.reciprocal(rr[:], tps[:, :, D:D + 1])
                nc.vector.tensor_tensor(
                    out=stage[:, :, h, :], in0=tps[:, :, 0:D],
                    in1=rr[:].unsqueeze(-1).broadcast_to([128, 4, D]),
                    op=mybir.AluOpType.mult)

            return output_path

        def start_stage():
            stage_ref[0] = spool.tile([128, 4, LH, D], F32, tag="stage",
                                      name="stage")

        def flush_stage(Q):
            st = stage_ref[0]
            return lambda: nc.sync.dma_start(
                out_v[Q, :, :, :], st[:].rearrange("p t h d -> p t (h d)"))

        def body():
            qk_all = []
            va_all = []
            carry = [None]

            def run_attn(h, Q):
                prev, carry[0] = carry[0], None
                carry[0] = emit_attn(h, Q, qk_all, va_all, carry=prev)

            for nb in range(NB):
                xt_sb = emit_xt(nb)
                va = vpool.tile([128, 4, LH, D + 1], MDT, tag="va")
                nc.vector.tensor_copy(va[:, :, :, D:D + 1], vones[:])
                qk_h = [qkpool.tile([D, 2, 512], MDT, tag=f"qk{h}",
                                    name=f"qk{h}")
                        for h in range(LH)]
                qk_all.append(qk_h)
                va_all.append(va)
                proj = ([lambda mt=mt: emit_vproj(xt_sb, va, mt)
                         for mt in range(4)]
                        + [lambda g=g: emit_qkg(xt_sb, qk_h, g)
                           for g in range(QKG)])
                if nb == 0:
                    for u in proj:
                        u()
                else:
                    attn = [lambda h=h: run_attn(h, nb - 1)
                            for h in range(LH)]
                    start_stage()
                    if ORDER == "attn_first":
                        order = [attn[0], proj[0], proj[4],
                                 attn[1], proj[1], proj[5],
                                 attn[2], proj[2], proj[6], proj[8],
                                 attn[3], proj[3], proj[7], proj[9]]
                    else:
                        order = [proj[0], proj[4], attn[0],
                                 proj[1], proj[5], attn[1],
                                 proj[2], proj[6], proj[7], attn[2],
                                 proj[3], proj[8], proj[9], attn[3]]
                    for u in order:
                        u()
                    # round's last output path + its stage flush ride along
                    # inside the next round's first attention unit
                    prev, fl = carry[0], flush_stage(nb - 1)

                    def tail_and_flush(prev=prev, fl=fl):
                        prev()
                        fl()
                    carry[0] = tail_and_flush
            start_stage()
            for h in range(LH):
                run_attn(h, NB - 1)
            carry[0]()
            flush_stage(NB - 1)()

        if reps == 1:
            body()
            if inline2:
                body()
        else:
            with tc.For_i(0, reps, 1):
                body()
                if UNROLL2:
                    body()

    nc.compile()
    return nc


def shard_inputs(x, W, b, use_f32r=None):
    """Full inputs -> per-core in_maps (bf16 for matmul operands)."""
    import ml_dtypes
    x = np.asarray(x, dtype=np.float32)
    W = np.asarray(W, dtype=np.float32)
    b = np.asarray(b, dtype=np.float32)
    rnd = lambda a: np.ascontiguousarray(
        np.asarray(a, dtype=np.float32).astype(ml_dtypes.bfloat16))
    in_maps = []
    for c in range(NCORES):
        bc, g = divmod(c, 2)
        h0 = g * LH
        # packed qk feature order per half t: QK_COL_ORDER runs
        qk_parts = []
        bqk_parts = []
        for t in range(2):
            for (h, r0, r1) in QK_COL_ORDER:
                c0 = t * C + (h0 + h) * D
                qk_parts.append(W[:, c0 + r0:c0 + r1])
                bqk_parts.append(b[c0 + r0:c0 + r1])
        vcols = [W[:, 2 * C + (h0 + h) * D:2 * C + (h0 + h + 1) * D]
                 for h in range(LH)]
        wqk = np.concatenate(qk_parts, axis=1)
        wv = np.concatenate(vcols, axis=1)
        # bias laid out to match the packed PSUM groups: bqk[p, grp]
        bq = np.concatenate(bqk_parts)
        bqk = np.ascontiguousarray(bq.reshape(QKG, 128).T)
        bv = np.concatenate(
            [b[2 * C + (h0 + h) * D:2 * C + (h0 + h + 1) * D]
             for h in range(LH)])[None, :]
        in_maps.append({
            "xt": rnd(x[bc].T),
            "wqk": rnd(wqk),
            "wv": rnd(wv),
            "bqk": np.ascontiguousarray(bqk, dtype=np.float32),
            "bv": rnd(bv),
        })
    return in_maps


def gather_outputs(results):
    """Per-core results -> full [B, N, C] output."""
    out = np.empty((B, N, C), dtype=np.float32)
    for c in range(NCORES):
        bc, g = divmod(c, 2)
        out[bc, :, g * OUTC:(g + 1) * OUTC] = results[c]["out"]
    return out


def kernel(x, W, b):
    nc = build(reps=1, use_f32r=USE_F32R)
    in_maps = shard_inputs(x, W, b, use_f32r=USE_F32R)
    res = run_bass_kernel_spmd(nc, in_maps, core_ids=list(range(NCORES)))
    return gather_outputs(res.results)


# revision 40
# speedup vs baseline: 2.0992x; 2.0992x over previous
"""Causal self-attention kernel for 8 Trainium2 NeuronCores.

Problem (hardcoded): x [4, 2048, 768] f32, W [768, 2304] f32, b [2304] f32.
reference: qkv = x@W+b; 8 heads, head_dim 96; causal softmax attention.

Sharding: core c handles batch c//2 and heads 4*(c%2) .. 4*(c%2)+3
(data-parallel over batch x tensor-parallel over heads). Host shards
inputs / gathers outputs around one SPMD NEFF; no device collectives.

v2 design (all matmul inputs bf16):
  - qk projection "packed": the core's 768 q|k output features are computed
    transposed in 6 full-128-row PSUM groups (vs 8 at 96 rows), then a
    12-op DVE evacuation (with per-partition bias) unpacks group rows into
    per-head qT/kT SBUF tiles [96, 2, 512].
  - v projection natural [seq, 4h, 96+1] with a ones column so the PV
    matmul also produces softmax denominators (bias via K=1 ones matmul).
  - attention in S^T layout, j-blocks of 128 k processed in groups of 4:
    4 matmuls -> one [128, <=2048] bf16 PSUM tile -> ONE wide exp ->
    (diag: 4 gpsimd causal mask selects) -> 4 PV matmuls accumulating
    o^T[d+1, 512] in fp32 PSUM. Diagonal group packs widths 512/384/256/128
    at offsets 0/512/1024/1280 so the exp covers 1408 cols, not 2048.
  - output: o^T PE-transposed back to natural [q, d] via bf16 transposes
    into one [128, 4, 128] PSUM tile, scaled by 1/denominator, DMA out.
  - emission interleaves projection units of block nb with attention units
    of block nb-1 so the scalar engine (exp) stays fed while the PE does
    projection work.
"""

import functools
from contextlib import ExitStack

import numpy as np

import concourse.bacc as bacc
import concourse.bass as bass
import concourse.mybir as mybir
import concourse.tile as tile
from concourse.bass_utils import run_bass_kernel_spmd
from concourse.masks import make_identity

F32 = mybir.dt.float32
F32R = mybir.dt.float32r
BF16 = mybir.dt.bfloat16

B, N, C, H = 4, 2048, 768, 8
D = C // H            # 96
NCORES = 8
LH = 4                # local heads per core
KC = C // 128         # 6 contraction chunks
NB = N // 512         # 4 seq blocks of 512
QKF = 2 * LH * D      # 768 packed q|k features per core
QKG = QKF // 128      # 6 psum groups
OUTC = LH * D         # 384
SCALE = float(1.0 / np.sqrt(np.float32(D)))
UNROLL2 = False
USE_F32R = False      # kept for compat; v2 is bf16-only
MASK_ENGINE = "gpsimd"
ORDER = "proj_first"  # proj_first | attn_first
PV_TRAIL = 2          # PV lags S/exp by this many j-groups
FUSED_MUL = False     # single broadcast tensor_tensor vs 4 tensor_scalar_mul

# attention processes j-blocks (128 k rows) in groups of 2 per [128, 1024]
# f32 PSUM tile (2 banks); each matmul output stays inside one 2KB bank.
# Each group entry: (j_rel_a, free_off, width); diagonal groups pack widths
# 512/384 and 256/128 tightly so the exp covers 896 / 384 cols.


# Packed qk projection: per half t (0=q, 1=k) the core's 4x96 features are
# computed in 3 full-128-row PSUM groups. Hardware partition-AP rule: an AP
# may start only at a 32-multiple and must not cross the next 64-boundary
# unless it starts at 0 (so (32, span>32) and (96, span>32) are illegal).
# The evacuation segments below are all legal because heads 1's rows are
# stored rotated (head-rows 0:32 at tile rows 64:96, head-rows 32:96 at tile
# rows 0:64) — any consistent permutation of the d-contraction is valid as
# long as q and k use the same one (they do: both halves share this layout).
# Segment: (group_offset, src_p0, src_p1, head, dst_r0); W columns are packed
# host-side in matching order (see shard_inputs).
QK_SEGMENTS_T = [
    (0, 0, 96, 0, 0),
    (0, 96, 128, 1, 64),
    (1, 0, 64, 1, 0),
    (1, 64, 128, 2, 0),
    (2, 0, 96, 3, 0),
    (2, 96, 128, 2, 64),
]

# per-head W column slices (r0, r1) in packed order for one t-half
QK_COL_ORDER = [(0, 0, 96), (1, 0, 32), (1, 32, 96), (2, 0, 64),
                (3, 0, 96), (2, 64, 96)]


@functools.lru_cache(maxsize=4)
def build(reps=1, use_f32r=False, inline2=False):
    assert not use_f32r, "v2 kernel is bf16-only"
    MDT = BF16
    nc = bacc.Bacc("TRN2", target_bir_lowering=False, debug=False,
                   num_devices=NCORES)
    xt_d = nc.dram_tensor("xt", [C, N], MDT, kind="ExternalInput")
    wqk_d = nc.dram_tensor("wqk", [C, QKF], MDT, kind="ExternalInput")
    wv_d = nc.dram_tensor("wv", [C, LH * D], MDT, kind="ExternalInput")
    bqk_d = nc.dram_tensor("bqk", [128, QKG], F32, kind="ExternalInput")
    bv_d = nc.dram_tensor("bv", [1, LH * D], MDT, kind="ExternalInput")
    out_d = nc.dram_tensor("out", [N, OUTC], F32, kind="ExternalOutput")

    xt_v = xt_d.ap().rearrange("(kc p) n -> p kc n", p=128)
    wqk_v = wqk_d.ap().rearrange("(kc p) m -> p kc m", p=128)
    wv_v = wv_d.ap().rearrange("(kc p) m -> p kc m", p=128)
    out_v = out_d.ap().rearrange("(qq t p) c -> qq p t c", t=4, p=128)

    with tile.TileContext(nc) as tc, ExitStack() as ctx:
        const = ctx.enter_context(tc.tile_pool(name="const", bufs=1))
        wpool = ctx.enter_context(tc.tile_pool(name="w", bufs=1))
        xpool = ctx.enter_context(tc.tile_pool(name="x", bufs=5))
        qkpool = ctx.enter_context(tc.tile_pool(name="qk", bufs=NB + 1))
        vpool = ctx.enter_context(tc.tile_pool(name="vaug", bufs=NB + 1))
        ppool = ctx.enter_context(tc.tile_pool(name="p", bufs=4))
        opool = ctx.enter_context(tc.tile_pool(name="osb", bufs=3))
        rpool = ctx.enter_context(tc.tile_pool(name="r", bufs=4))
        spool = ctx.enter_context(tc.tile_pool(name="stage", bufs=3))
        ps_proj = ctx.enter_context(
            tc.tile_pool(name="ps_proj", bufs=2, space="PSUM"))
        ps_s = ctx.enter_context(
            tc.tile_pool(name="ps_s", bufs=2, space="PSUM"))
        ps_o = ctx.enter_context(
            tc.tile_pool(name="ps_o", bufs=2, space="PSUM"))
        ps_t = ps_proj  # transposes share the proj banks (same tag "proj")

        # one-time constants
        identity = const.tile([128, 128], MDT)
        make_identity(nc, identity[:])
        ones = const.tile([1, 128], F32)
        nc.gpsimd.memset(ones[:], 1.0)
        ones_r = const.tile([1, 128], MDT)
        nc.vector.tensor_copy(ones_r[:], ones[:])
        vones = const.tile([128, 4, LH, 1], F32)
        nc.gpsimd.memset(vones[:], 1.0)

        stage_ref = [None]
        wqk_sb = wpool.tile([128, KC, QKF], MDT, tag="wqk")
        wv_sb = wpool.tile([128, KC, LH * D], MDT, tag="wv")
        for kc in range(KC):
            nc.sync.dma_start(wqk_sb[:, kc, :], wqk_v[:, kc, :])
            nc.sync.dma_start(wv_sb[:, kc, :], wv_v[:, kc, :])
        bqk_sb = wpool.tile([128, QKG], F32, tag="bqk")
        nc.sync.dma_start(bqk_sb[:], bqk_d.ap())
        bv_sb = wpool.tile([1, LH * D], MDT, tag="bv")
        nc.sync.dma_start(bv_sb[:], bv_d.ap())

        def emit_xt(nb):
            xt_sb = xpool.tile([128, KC, 512], MDT, tag="xt")
            for kc in range(KC):
                nc.sync.dma_start(
                    xt_sb[:, kc, :], xt_v[:, kc, nb * 512:(nb + 1) * 512])
            return xt_sb

        def emit_vproj(xt_sb, va, mt):
            vps = ps_proj.tile([128, 512], F32, tag="proj")
            nc.tensor.matmul(vps[:, 0:LH * D], ones_r[:, :], bv_sb[:, :],
                             start=True, stop=False)
            for kc in range(KC):
                nc.tensor.matmul(
                    vps[:, 0:LH * D],
                    xt_sb[:, kc, mt * 128:(mt + 1) * 128],
                    wv_sb[:, kc, :],
                    start=False, stop=(kc == KC - 1))
            nc.vector.tensor_copy(
                va[:, mt, :, 0:D],
                vps[:, 0:LH * D].rearrange("p (h d) -> p h d", h=LH))

        def emit_qkg(xt_sb, qk_h, g):
            qps = ps_proj.tile([128, 512], F32, tag="proj")
            for kc in range(KC):
                nc.tensor.matmul(
                    qps[:, :],
                    wqk_sb[:, kc, 128 * g:128 * g + 128],
                    xt_sb[:, kc, :],
                    start=(kc == 0), stop=(kc == KC - 1))
            t, grel = divmod(g, QKG // 2)
            for (go, p0, p1, h, r0) in QK_SEGMENTS_T:
                if go != grel:
                    continue
                nc.vector.tensor_scalar_add(
                    qk_h[h][r0:r0 + (p1 - p0), t, :],
                    qps[p0:p1, :],
                    bqk_sb[p0:p1, g:g + 1])

        def emit_pv(ops, h, ptinfo, va_all, first, last):
            pt, entries, diag = ptinfo
            n = len(entries)
            for i, (j, off, w) in enumerate(entries):
                qcol = 128 * (j % 4) if diag else 0
                nc.tensor.matmul(
                    ops[:, qcol:512],
                    va_all[j // 4][:, j % 4, h, :],
                    pt[:, off:off + w],
                    start=(first and i == 0), stop=(last and i == n - 1))

        def emit_attn(h, Q, qk_all, va_all, carry=None):
            ops = ps_o.tile([D + 1, 512], F32, tag="o")
            # diagonal groups first: their exp -> mask -> PV latency then
            # hides under the off-diagonal groups' matmuls
            groups = [
                (True, [(4 * Q, 0, 512), (4 * Q + 1, 512, 384)]),
                (True, [(4 * Q + 2, 0, 256), (4 * Q + 3, 256, 128)]),
            ]
            for g in range(Q):
                groups.append((False, [(4 * g, 0, 512), (4 * g + 1, 512, 512)]))
                groups.append((False, [(4 * g + 2, 0, 512),
                                       (4 * g + 3, 512, 512)]))
            pts = []
            for gi, (diag, entries) in enumerate(groups):
                sps = ps_s.tile([128, 1024], F32, tag="s")
                for (j, off, w) in entries:
                    qrel = 128 * (j % 4) if diag else 0
                    nc.tensor.matmul(
                        sps[:, off:off + w],
                        qk_all[j // 4][h][:, 1, 128 * (j % 4):
                                          128 * (j % 4) + 128],
                        qk_all[Q][h][:, 0, qrel:512],
                        start=True, stop=True)
                jl, offl, wl = entries[-1]
                ew = offl + wl
                pt = ppool.tile([128, 1024], MDT, tag="p")
                nc.scalar.activation(
                    pt[:, 0:ew], sps[:, 0:ew],
                    mybir.ActivationFunctionType.Exp, scale=SCALE)
                if diag:
                    for (j, off, w) in entries:
                        nc.gpsimd.affine_select(
                            out=pt[:, off:off + 128],
                            in_=pt[:, off:off + 128],
                            compare_op=mybir.AluOpType.is_ge,
                            fill=0.0, base=0, pattern=[[1, 128]],
                            channel_multiplier=-1)
                pts.append((pt, entries, diag))
                if gi == 0 and carry is not None:
                    carry()  # previous unit's output path, behind our S(0)
                if gi >= PV_TRAIL:
                    emit_pv(ops, h, pts[gi - PV_TRAIL], va_all,
                            first=(gi - PV_TRAIL == 0), last=False)
            ng = len(groups)
            for gi in range(max(0, ng - PV_TRAIL), ng):
                emit_pv(ops, h, pts[gi], va_all,
                        first=(gi == 0), last=(gi == ng - 1))

            # ---- output: transpose, normalize into the per-Q staging ----
            o_sb = opool.tile([D + 1, 512], MDT, tag="osb")
            nc.vector.tensor_copy(o_sb[:], ops[:])
            stage = stage_ref[0]

            def output_path():
                tps = ps_t.tile([128, 4, 128], MDT, tag="proj")
                for t in range(4):
                    nc.tensor.transpose(
                        tps[:, t, 0:D + 1], o_sb[:, t * 128:(t + 1) * 128],
                        identity[0:D + 1, 0:D + 1])
                rr = rpool.tile([128, 4], F32, tag="r")
                nc.vector.reciprocal(rr[:], tps[:, :, D:D + 1])
                if FUSED_MUL:
                    nc.vector.tensor_tensor(
                        out=stage[:, :, h, :], in0=tps[:, :, 0:D],
                        in1=rr[:].unsqueeze(-1).broadcast_to([128, 4, D]),
                        op=mybir.AluOpType.mult)
                else:
                    for t in range(4):
                        nc.vector.tensor_scalar_mul(
                            stage[:, t, h, :], tps[:, t, 0:D], rr[:, t:t + 1])

            return output_path

        def start_stage():
            stage_ref[0] = spool.tile([128, 4, LH, D], F32, tag="stage",
                                      name="stage")

        def flush_stage(Q):
            st = stage_ref[0]
            return lambda: nc.sync.dma_start(
                out_v[Q, :, :, :], st[:].rearrange("p t h d -> p t (h d)"))

        def body():
            qk_all = []
            va_all = []
            carry = [None]

            def run_attn(h, Q):
                prev, carry[0] = carry[0], None
                carry[0] = emit_attn(h, Q, qk_all, va_all, carry=prev)

            for nb in range(NB):
                xt_sb = emit_xt(nb)
                va = vpool.tile([128, 4, LH, D + 1], MDT, tag="va")
                nc.vector.tensor_copy(va[:, :, :, D:D + 1], vones[:])
                qk_h = [qkpool.tile([D, 2, 512], MDT, tag=f"qk{h}",
                                    name=f"qk{h}")
                        for h in range(LH)]
                qk_all.append(qk_h)
                va_all.append(va)
                proj = ([lambda mt=mt: emit_vproj(xt_sb, va, mt)
                         for mt in range(4)]
                        + [lambda g=g: emit_qkg(xt_sb, qk_h, g)
                           for g in range(QKG)])
                if nb == 0:
                    for u in proj:
                        u()
                else:
                    attn = [lambda h=h: run_attn(h, nb - 1)
                            for h in range(LH)]
                    start_stage()
                    if ORDER == "attn_first":
                        order = [attn[0], proj[0], proj[4],
                                 attn[1], proj[1], proj[5],
                                 attn[2], proj[2], proj[6], proj[8],
                                 attn[3], proj[3], proj[7], proj[9]]
                    else:
                        order = [proj[0], proj[4], attn[0],
                                 proj[1], proj[5], attn[1],
                                 proj[2], proj[6], proj[7], attn[2],
                                 proj[3], proj[8], proj[9], attn[3]]
                    for u in order:
                        u()
                    # round's last output path + its stage flush ride along
                    # inside the next round's first attention unit
                    prev, fl = carry[0], flush_stage(nb - 1)

                    def tail_and_flush(prev=prev, fl=fl):
                        prev()
                        fl()
                    carry[0] = tail_and_flush
            start_stage()
            for h in range(LH):
                run_attn(h, NB - 1)
            carry[0]()
            flush_stage(NB - 1)()

        if reps == 1:
            body()
            if inline2:
                body()
        else:
            with tc.For_i(0, reps, 1):
                body()
                if UNROLL2:
                    body()

    nc.compile()
    return nc


def shard_inputs(x, W, b, use_f32r=None):
    """Full inputs -> per-core in_maps (bf16 for matmul operands)."""
    import ml_dtypes
    x = np.asarray(x, dtype=np.float32)
    W = np.asarray(W, dtype=np.float32)
    b = np.asarray(b, dtype=np.float32)
    rnd = lambda a: np.ascontiguousarray(
        np.asarray(a, dtype=np.float32).astype(ml_dtypes.bfloat16))
    in_maps = []
    for c in range(NCORES):
        bc, g = divmod(c, 2)
        h0 = g * LH
        # packed qk feature order per half t: QK_COL_ORDER runs
        qk_parts = []
        bqk_parts = []
        for t in range(2):
            for (h, r0, r1) in QK_COL_ORDER:
                c0 = t * C + (h0 + h) * D
                qk_parts.append(W[:, c0 + r0:c0 + r1])
                bqk_parts.append(b[c0 + r0:c0 + r1])
        vcols = [W[:, 2 * C + (h0 + h) * D:2 * C + (h0 + h + 1) * D]
                 for h in range(LH)]
        wqk = np.concatenate(qk_parts, axis=1)
        wv = np.concatenate(vcols, axis=1)
        # bias laid out to match the packed PSUM groups: bqk[p, grp]
        bq = np.concatenate(bqk_parts)
        bqk = np.ascontiguousarray(bq.reshape(QKG, 128).T)
        bv = np.concatenate(
            [b[2 * C + (h0 + h) * D:2 * C + (h0 + h + 1) * D]
             for h in range(LH)])[None, :]
        in_maps.append({
            "xt": rnd(x[bc].T),
            "wqk": rnd(wqk),
            "wv": rnd(wv),
            "bqk": np.ascontiguousarray(bqk, dtype=np.float32),
            "bv": rnd(bv),
        })
    return in_maps


def gather_outputs(results):
    """Per-core results -> full [B, N, C] output."""
    out = np.empty((B, N, C), dtype=np.float32)
    for c in range(NCORES):
        bc, g = divmod(c, 2)
        out[bc, :, g * OUTC:(g + 1) * OUTC] = results[c]["out"]
    return out


def kernel(x, W, b):
    nc = build(reps=1, use_f32r=USE_F32R)
    in_maps = shard_inputs(x, W, b, use_f32r=USE_F32R)
    res = run_bass_kernel_spmd(nc, in_maps, core_ids=list(range(NCORES)))
    return gather_outputs(res.results)
